# revision 32
# baseline (speedup 1.0000x reference)
"""Masked dot-product attention (B=8, Q=K=2048, D=64) for 8 NeuronCores.

Strategy:
  - Shard the query dim across the 8 cores (256 queries per core, all 8
    batches on every core).  Unlike batch-sharding this is perfectly
    load-balanced for any distribution of valid_lens.
  - kernel() reads valid_lens on the host and compiles a Bass program
    specialized to those lengths: per batch only ceil(L/128) key tiles are
    loaded/computed; the masked tail gets the reference's -1e6 fill via a
    per-partition bias add before exp (underflows to 0 in f32).
  - Scores are computed transposed, S^T[k, q], with the head dim (64) on
    partitions; two batches are packed into the two 64-row groups of the
    PE array (tile_position row packing).  float32r operands run the PE at
    1 column/cycle (plain f32 would cost 4).
  - Softmax skips the max-subtraction (scores are bounded: q,k ~ N(0,1),
    score = q.k/8, far below exp overflow).  exp runs on the scalar engine
    straight out of PSUM over multi-bank spans to amortize overhead.
  - PV uses out^T[d, q] = sum_k V'[k, d] * P^T[k, q] with V' = [V | 1]
    (ones column built on the host): row 64 of the accumulator is the
    softmax denominator for free.
  - Epilogue: tiny PE transposes of [65, 128] chunks + reciprocal +
    tensor_scalar multiply produce normalized [q, d] output tiles.
  - Q^T and K^T are concatenated into one host tensor per batch so each
    pair-half is loaded by a single DMA (keeps matmul wait fan-in low).
"""

import os
import sys

import numpy as np

for _p in ("/opt/trn_rl_repo", "/root/.axon_site/_ro/trn_rl_repo"):
    if os.path.isdir(_p) and _p not in sys.path:
        sys.path.insert(0, _p)

B, Q, K, D = 8, 2048, 2048, 64
N_CORES = 8
QC = Q // N_CORES  # queries per core
KT = 128           # key-tile size (k rows per S^T tile)
GROUP = 4          # S^T tiles per PSUM exp group (2 banks of 2)


def _build_nc(Ls):
    """Build the Bass program, specialized to the list of valid lengths."""
    import concourse.bass as bass
    import concourse.mybir as mybir
    import concourse.tile as tile
    from concourse.masks import make_identity

    f32 = mybir.dt.float32
    f32r = mybir.dt.float32r

    nt = [(int(L) + KT - 1) // KT for L in Ls]   # k-tiles per batch
    lmod = [int(L) % KT for L in Ls]             # valid rows in last tile (0 = full)

    # Pair batches (largest with next largest) to balance the row-packed
    # S^T matmuls; process pairs longest-first so DMA prefetch lines up.
    order = sorted(range(B), key=lambda b: -nt[b])
    pairs = [(order[2 * i], order[2 * i + 1]) for i in range(B // 2)]

    nc = bass.Bass()
    # kq[p] = per-pair interleave: rows 0-63 = batch a's [Q^T_slice | K^T],
    # rows 64-127 = batch b's.  One full-partition DMA per pair.
    kq_d = nc.dram_tensor("kq", [B // 2, 128, QC + K], f32r, kind="ExternalInput")
    # v ships partition-major: v[b, p, t, :] = V'[b, t*KT + p, :] so each
    # per-partition DMA run is nt*260 bytes contiguous (full DMA bandwidth)
    v_d = nc.dram_tensor("v", [B, KT, K // KT, D + 1], f32r, kind="ExternalInput")
    mb_d = nc.dram_tensor("maskb", [128, B], f32, kind="ExternalInput")
    # out is partition-major too: out[b, p, h, :] = O[b, h*128 + p, :]
    out_d = nc.dram_tensor("out", [B, 128, QC // 128, D], f32, kind="ExternalOutput")

    with tile.TileContext(nc) as tc:
        with (
            tc.tile_pool(name="persist", bufs=1) as persist,
            tc.tile_pool(name="pt", bufs=6) as pt_pool,
            tc.tile_pool(name="psum", bufs=1, space="PSUM") as psum_pool,
            tc.tile_pool(name="oT", bufs=4) as ot_pool,
            tc.tile_pool(name="osb", bufs=4) as osb_pool,
            tc.tile_pool(name="rec", bufs=6) as rec_pool,
        ):
            ident = persist.tile([128, 128], f32, tag="ident")
            make_identity(nc, ident)
            maskb = persist.tile([128, B], f32, tag="maskb")
            nc.sync.dma_start(out=maskb, in_=mb_d[:, :])
            # touch Exp immediately so the ~2.7us ACT table load overlaps
            # the initial DMAs instead of stalling the first softmax
            warm = persist.tile([128, 1], f32, tag="warm")
            nc.vector.memset(warm, 0.0)
            nc.scalar.activation(
                out=warm, in_=warm, func=mybir.ActivationFunctionType.Exp
            )

            # ---- persistent input buffers + DMA loads (processing order) ----
            kq_sb = {}
            v_sb = {}
            for p, (a, b) in enumerate(pairs):
                width = QC + max(nt[a], nt[b]) * KT
                kqs = persist.tile([128, width], f32r, tag=f"kq{p}")
                kq_sb[p] = kqs
                # chunked load: first chunk (q + 8 k-tiles) unblocks the
                # first matmul groups early; the rest streams behind
                edges = [0] + list(range(QC + 8 * KT, width, 8 * KT)) + [width]
                for e0, e1 in zip(edges[:-1], edges[1:]):
                    nc.sync.dma_start(
                        out=kqs[:, e0:e1], in_=kq_d[p][:, e0:e1]
                    )
                for bi in (a, b):
                    vs = persist.tile([128, nt[bi], D + 1], f32r, tag=f"v{bi}")
                    v_sb[bi] = vs
                    nc.sync.dma_start(out=vs, in_=v_d[bi][:, : nt[bi], :])

            # PE warm-up: keep the array busy while the first DMAs land so
            # the first real matmuls run at full clock
            wps = psum_pool.tile([128, 128], f32, tag="tp", name="wps", bufs=2)
            for _ in range(12):
                nc.tensor.matmul(wps, lhsT=ident, rhs=ident, start=True, stop=True)

            # ---- main pipeline ----
            for p, (a, b) in enumerate(pairs):
                # one PSUM bank per accumulator: concurrent accumulation
                # groups must not share a bank (group start clears the
                # whole bank's has_written bits)
                accs = {
                    0: psum_pool.tile([128, 256], f32, tag="accA", name="accA"),
                    1: psum_pool.tile([128, 256], f32, tag="accB", name="accB"),
                }
                # Each group covers k-steps {g0, g0+1} for both halves.
                # Half 0's tiles go to bank 0 (slots 0,1), half 1's to bank 1
                # (slots 2,3): the two concurrently-executing row-group
                # matmuls of a k-step must not write the same PSUM bank.
                for g0 in range(0, max(nt[a], nt[b]), 2):
                    sp = psum_pool.tile(
                        [128, GROUP * QC], f32, tag="spsum", name="spsum", bufs=2
                    )
                    ptile = pt_pool.tile(
                        [128, GROUP * QC], f32r, tag="pt", name="ptile"
                    )
                    entries = []
                    for half, bi in ((0, a), (1, b)):
                        for kt in (g0, g0 + 1):
                            if kt >= nt[bi]:
                                continue
                            s = 2 * half + (kt - g0)
                            sl = slice(64 * half, 64 * half + 64)
                            nc.tensor.matmul(
                                sp[:, s * QC : (s + 1) * QC],
                                lhsT=kq_sb[p][sl, QC + kt * KT : QC + (kt + 1) * KT],
                                rhs=kq_sb[p][sl, 0:QC],
                                start=True,
                                stop=True,
                                tile_position=(64 * half, 0),
                            )
                            entries.append((bi, half, kt, s))
                    # exp: one merged call when all four slots are normal,
                    # else per-half contiguous runs + biased boundary singles
                    runs = []          # (slot_start, n_tiles)
                    singles = []       # (slot, batch) -> biased exp
                    for half, bi in ((0, a), (1, b)):
                        tiles_h = [e for e in entries if e[1] == half]
                        normal = [
                            e for e in tiles_h
                            if not (e[2] == nt[bi] - 1 and lmod[bi] != 0)
                        ]
                        bdry = [
                            e for e in tiles_h
                            if (e[2] == nt[bi] - 1 and lmod[bi] != 0)
                        ]
                        if normal:
                            runs.append((2 * half, len(normal)))
                        for e in bdry:
                            singles.append((e[3], bi))
                    if runs == [(0, 2), (2, 2)]:
                        runs = [(0, 4)]
                    for s0, ntile in runs:
                        nc.scalar.activation(
                            out=ptile[:, s0 * QC : (s0 + ntile) * QC],
                            in_=sp[:, s0 * QC : (s0 + ntile) * QC],
                            func=mybir.ActivationFunctionType.Exp,
                        )
                    for s, bi in singles:
                        nc.scalar.activation(
                            out=ptile[:, s * QC : (s + 1) * QC],
                            in_=sp[:, s * QC : (s + 1) * QC],
                            func=mybir.ActivationFunctionType.Exp,
                            bias=maskb[:, bi : bi + 1],
                        )
                    for bi, half, kt, s in entries:
                        nc.tensor.matmul(
                            accs[half][0 : D + 1, :],
                            lhsT=v_sb[bi][:, kt, :],
                            rhs=ptile[:, s * QC : (s + 1) * QC],
                            start=(kt == 0),
                            stop=(kt == nt[bi] - 1),
                        )
                        if kt == nt[bi] - 1:
                            # epilogue as soon as this half's accumulation ends
                            oT = ot_pool.tile([D + 1, QC], f32, tag="oT", name="oT")
                            nc.vector.tensor_copy(
                                out=oT, in_=accs[half][0 : D + 1, :]
                            )
                            osb = osb_pool.tile(
                                [128, QC // 128, D], f32, tag="osb", name="osb"
                            )
                            for qh in range(QC // 128):
                                tp = psum_pool.tile(
                                    [128, D + 1], f32, tag="tp", name="tp", bufs=2
                                )
                                nc.tensor.transpose(
                                    tp,
                                    oT[:, qh * 128 : (qh + 1) * 128],
                                    ident[0 : D + 1, 0 : D + 1],
                                )
                                rec = rec_pool.tile([128, 1], f32, tag="rec", name="rec")
                                nc.vector.reciprocal(rec, tp[:, D : D + 1])
                                nc.vector.tensor_scalar_mul(
                                    osb[:, qh, :], tp[:, 0:D], rec
                                )
                            nc.sync.dma_start(out=out_d[bi], in_=osb)

                # ---- epilogue: transpose + normalize + store ----
                for half, bi in ((0, a), (1, b)):
                    oT = ot_pool.tile([D + 1, QC], f32, tag="oT")
                    nc.vector.tensor_copy(out=oT, in_=accs[half][0 : D + 1, :])
                    for qh in range(QC // 128):
                        tp = tp_pool.tile([128, D + 1], f32, tag="tp")
                        nc.tensor.transpose(
                            tp,
                            oT[:, qh * 128 : (qh + 1) * 128],
                            ident[0 : D + 1, 0 : D + 1],
                        )
                        rec = rec_pool.tile([128, 1], f32, tag="rec")
                        nc.vector.reciprocal(rec, tp[:, D : D + 1])
                        osb = osb_pool.tile([128, D], f32, tag="osb")
                        nc.vector.tensor_scalar_mul(osb, tp[:, 0:D], rec)
                        nc.sync.dma_start(
                            out=out_d[bi][qh * 128 : (qh + 1) * 128, :], in_=osb
                        )

    # walrus codegen accepts at most one sync wait per engine instruction;
    # split the extras into EventSemaphore instructions (same pass bacc runs).
    import bass_rust

    bass_rust.generate_event_semaphores(nc)
    return nc


def kernel(queries, keys, values, valid_lens):
    return kernel_ex(queries, keys, values, valid_lens)[0]


def kernel_ex(queries, keys, values, valid_lens, trace=False):
    from concourse.bass_utils import run_bass_kernel_spmd

    Ls = [int(x) for x in np.asarray(valid_lens).reshape(-1)]
    assert len(Ls) == B

    # Host-side prep: scale Q by 1/sqrt(D), pre-transpose Q and K (f32 DMA
    # transpose is unsupported), append the ones column to V.
    q = np.ascontiguousarray(queries, dtype=np.float32) * np.float32(1.0 / np.sqrt(D))
    qt = np.ascontiguousarray(q.transpose(0, 2, 1))                  # [B, D, Q]
    kt = np.ascontiguousarray(
        np.asarray(keys, dtype=np.float32).transpose(0, 2, 1)
    )                                                                # [B, D, K]
    v1 = np.ones((B, K, D + 1), dtype=np.float32)
    v1[:, :, :D] = np.asarray(values, dtype=np.float32)              # [B, K, D+1]
    # partition-major relayout: [B, K, D+1] -> [B, KT, K//KT, D+1]
    v1 = np.ascontiguousarray(
        v1.reshape(B, K // KT, KT, D + 1).transpose(0, 2, 1, 3)
    )

    # mask bias columns: 0 where the key row of the last tile is valid,
    # -1e6 where it must be masked (matches the reference fill value)
    maskb = np.zeros((128, B), dtype=np.float32)
    for b in range(B):
        lm = Ls[b] % KT
        if lm:
            maskb[lm:, b] = np.float32(-1e6)

    # replicate the pairing logic of _build_nc to lay out the kq tensor
    nt = [(L + KT - 1) // KT for L in Ls]
    order = sorted(range(B), key=lambda b: -nt[b])
    pairs = [(order[2 * i], order[2 * i + 1]) for i in range(B // 2)]

    nc = _build_nc(Ls)
    in_maps = []
    for c in range(N_CORES):
        kq = np.zeros((B // 2, 128, QC + K), dtype=np.float32)
        for p, (a, b) in enumerate(pairs):
            for half, bi in ((0, a), (1, b)):
                kq[p, 64 * half : 64 * half + 64, :QC] = qt[
                    bi, :, c * QC : (c + 1) * QC
                ]
                kq[p, 64 * half : 64 * half + 64, QC:] = kt[bi]
        in_maps.append({"kq": np.ascontiguousarray(kq), "v": v1, "maskb": maskb})
    res = run_bass_kernel_spmd(
        nc, in_maps, core_ids=list(range(N_CORES)), trace=trace
    )

    out = np.empty((B, Q, D), dtype=np.float32)
    for c in range(N_CORES):
        # [B, 128, QC//128, D] -> [B, QC, D]
        o = res.results[c]["out"].transpose(0, 2, 1, 3).reshape(B, QC, D)
        out[:, c * QC : (c + 1) * QC, :] = o
    return out, res
